# revision 46
# baseline (speedup 1.0000x reference)
"""Trainium2 Bass kernel for nn_DynamicGCNModel (2-layer GCN+GRU, 50k nodes,
1.6M edges, C=128) on 8 NeuronCores.

Sharding: nodes split 6272/core (dim 0), edges partitioned by destination
window (128 nodes); per-edge source rows fetched with dma_gather from a
node-major bf16 table (replicated via AllGather), segment-summed with a
one-hot matmul into PSUM. Dense per-node math is feature-major.

v2 changes vs baseline:
- t_embed via the scalar-engine Sin activation (range-reduced on DVE)
- dinv broadcasts shipped from host (no on-device transposes)
- outputs written feature-major, transposed on host
- conv epilogues moved to the scalar engine (frees DVE for one-hots,
  reduces DVE<->GpSimd SBUF port contention during SWDGE desc-gen)
- gathers issued as prepare_only + trigger waves so Q7 descriptor
  generation overlaps the table AllGather and earlier windows
"""

import os

import numpy as np
import ml_dtypes

import concourse.bass as bass
import concourse.bacc as bacc
import concourse.mybir as mybir
import concourse.tile as tile
from concourse.bass_utils import run_bass_kernel_spmd

BF = ml_dtypes.bfloat16
F32 = mybir.dt.float32
BF16 = mybir.dt.bfloat16
I16 = mybir.dt.int16
I32 = mybir.dt.int32
AL = mybir.AluOpType
AF = mybir.ActivationFunctionType

N = 50000
NV = 50176
C = 128
NCORES = 8
NLOC = NV // NCORES   # 6272
NW = NLOC // 128      # 49
HALF = 25088
PAD_DST = 200.0
PRE = int(os.environ.get("K_PRE", "5"))     # windows gathered ahead
GATHER_PREP = os.environ.get("K_PREP", "0") == "1"
FUSE = os.environ.get("K_FUSE", "0") == "1"
NEGPAD = os.environ.get("K_NEGPAD", "0") == "1"
OHTS = os.environ.get("K_OHTS", "0") == "1"   # one-hot via tensor_scalar
DIRECT = os.environ.get("K_DIRECT", "1") == "1"  # gather from Shared table

LAST_EXEC_NS = None


# ---------------------------------------------------------------------------
# host preprocessing
# ---------------------------------------------------------------------------

def _hilo(a):
    a = np.asarray(a, np.float32)
    hi = a.astype(BF)
    lo = (a - hi.astype(np.float32)).astype(BF)
    return np.stack([hi, lo], 0)


def _preprocess(inp):
    src = np.asarray(inp["edge_index"][0], np.int64)
    dst = np.asarray(inp["edge_index"][1], np.int64)
    loops = np.arange(N, dtype=np.int64)
    src = np.concatenate([src, loops])
    dst = np.concatenate([dst, loops])

    deg = np.bincount(dst, minlength=NV).astype(np.float64)
    dinv = np.where(deg > 0, 1.0 / np.sqrt(np.maximum(deg, 1e-12)), 0.0)

    core = dst // NLOC
    win = (dst % NLOC) // 128
    dstl = (dst % 128).astype(np.float32)
    half = (src >= HALF).astype(np.int64)
    idx16 = (src - half * HALF).astype(np.int64)

    order = np.lexsort((half, win, core))
    core_s, win_s, dstl_s, half_s, idx_s = (
        core[order], win[order], dstl[order], half[order], idx16[order])

    key = (core_s * NW + win_s) * 2 + half_s
    nkeys = NCORES * NW * 2
    cnt = np.bincount(key, minlength=nkeys).reshape(NCORES, NW, 2)
    t_lo = int(np.ceil(cnt[:, :, 0].max() / 128))
    t_hi = int(np.ceil(cnt[:, :, 1].max() / 128))
    T = t_lo + t_hi

    starts = np.zeros(nkeys + 1, np.int64)
    np.cumsum(cnt.reshape(-1), out=starts[1:])
    pos_in_key = np.arange(len(key)) - starts[key]

    per_core = []
    for k in range(NCORES):
        # -1 pads: trailing negative indices are trimmed by the gather
        # ucode (fewer descriptors); slots stay stale in SBUF but the
        # one-hot zeroes their contribution (PAD_DST never matches iota).
        fill = -1 if NEGPAD else 0
        idx_arr = np.full((16, NW * T * 8), fill, np.int16)
        dstl_arr = np.full((128, NW * T), PAD_DST, BF)
        sel = core_s == k
        w = win_s[sel]
        h = half_s[sel]
        p = pos_in_key[sel]
        iv = idx_s[sel]
        dv = dstl_s[sel]
        callbase8 = (w * T + h * t_lo) * 8
        idx_arr[p % 16, callbase8 + p // 16] = iv.astype(np.int16)
        tcol = w * T + h * t_lo + p // 128
        dstl_arr[p % 128, tcol] = dv.astype(BF)
        per_core.append(dict(
            idx_all=np.tile(idx_arr, (8, 1)),
            dstl_all=dstl_arr,
        ))

    nfp = np.zeros((NV, C), np.float32)
    nfp[:N] = np.asarray(inp["node_features"], np.float32)
    ts_p = np.zeros(NV, np.float32)
    ts_p[:N] = np.asarray(inp["ts"], np.float32).reshape(-1)
    xp1 = np.zeros((NV, C), np.float32)
    xp1[:N] = np.asarray(inp["x_prev1"], np.float32)
    xp2 = np.zeros((NV, C), np.float32)
    xp2[:N] = np.asarray(inp["x_prev2"], np.float32)

    freq = np.asarray(inp["basis_freq"], np.float64)
    freq2 = (freq / (2 * np.pi)).astype(np.float32)
    phase2 = np.asarray(inp["phase"], np.float64) / (2 * np.pi)

    mW = np.asarray(inp["merge_W"], np.float64)
    W1_ = np.asarray(inp["W1"], np.float64)
    W2_ = np.asarray(inp["W2"], np.float64)
    sW = np.asarray(inp["skip_W"], np.float64)
    M1 = mW.T @ W1_.T
    S1 = mW.T @ sW.T
    b_m = np.asarray(inp["merge_b"], np.float64)

    static = dict(t_lo=t_lo, t_hi=t_hi, T=T)
    consts = dict(
        R1a=M1[:C].astype(BF), R1b=M1[C:].astype(BF),
        S1a=S1[:C].astype(BF), S1b=S1[C:].astype(BF),
        W2T=W2_.T.astype(BF),
        tab1_bias2=_hilo(b_m @ W1_.T).reshape(2, C),
        skip_bias2=_hilo(b_m @ sW.T +
                         np.asarray(inp["skip_b"], np.float64)).reshape(2, C),
        b1_col=np.asarray(inp["b1"], np.float32).reshape(C, 1),
        b2_col=np.asarray(inp["b2"], np.float32).reshape(C, 1),
        iota=np.tile(np.arange(128, dtype=np.float32).astype(BF), (128, 1)),
        freq2_col=freq2.reshape(C, 1),
        pc_col=(phase2 + 0.25).astype(np.float32).reshape(C, 1),
    )
    for l in (1, 2):
        Wih = np.asarray(inp[f"gru{l}_Wih"], np.float32)
        Whh = np.asarray(inp[f"gru{l}_Whh"], np.float32)
        bih = np.asarray(inp[f"gru{l}_bih"], np.float32)
        bhh = np.asarray(inp[f"gru{l}_bhh"], np.float32)
        for gi, gate in enumerate("rzn"):
            consts[f"g{l}Wi{gate}"] = Wih[gi * C:(gi + 1) * C].T.astype(BF)
            consts[f"g{l}Wh{gate}"] = Whh[gi * C:(gi + 1) * C].T.astype(BF)
        consts[f"g{l}brz_r"] = (bih[0:C] + bhh[0:C]).reshape(C, 1)
        consts[f"g{l}brz_z"] = (bih[C:2 * C] + bhh[C:2 * C]).reshape(C, 1)
        consts[f"g{l}bin"] = bih[2 * C:].reshape(C, 1)
        consts[f"g{l}bhn"] = bhh[2 * C:].reshape(C, 1)

    for k in range(NCORES):
        lo, hi_ = k * NLOC, (k + 1) * NLOC
        d = per_core[k]
        d["nf_fm"] = np.ascontiguousarray(nfp[lo:hi_].T.astype(BF))
        d["ts_rep"] = np.ascontiguousarray(
            np.tile(ts_p[lo:hi_], (128, 1)).astype(np.float32))
        d["xp1_fm"] = np.ascontiguousarray(xp1[lo:hi_].T)
        d["xp1_fmb"] = np.ascontiguousarray(xp1[lo:hi_].T.astype(BF))
        d["xp2_fm"] = np.ascontiguousarray(xp2[lo:hi_].T)
        d["xp2_fmb"] = np.ascontiguousarray(xp2[lo:hi_].T.astype(BF))
        dloc = dinv[lo:hi_]
        d["dinv_nm"] = np.ascontiguousarray(
            dloc.reshape(NW, 128).T.astype(np.float32))
        d["dinvb_fm"] = np.ascontiguousarray(
            np.tile(dloc, (128, 1)).astype(BF))
        mask = np.zeros((1, NLOC), np.float32)
        mask[0, :max(0, min(NLOC, N - lo))] = 1.0
        d["mask_row"] = mask.astype(BF)
        d.update(consts)
    return per_core, static


# ---------------------------------------------------------------------------
# bass program
# ---------------------------------------------------------------------------

def _bcast_free(ap_2d, cnt_mid, cnt_inner, mode):
    """3D broadcast AP from a 2D slice.

    mode 'rep_elem': [p, m] -> [p, m, inner] repeating each element
    mode 'rep_row':  [p, inner] -> [p, mid, inner] repeating the row
    """
    if mode == "rep_elem":
        return bass.AP(ap_2d.tensor, ap_2d.offset,
                       [ap_2d.ap[0], [1, cnt_mid], [0, cnt_inner]])
    return bass.AP(ap_2d.tensor, ap_2d.offset,
                   [ap_2d.ap[0], [0, cnt_mid], [1, cnt_inner]])


def _build(nc, static):
    PH = int(os.environ.get("K_PH", "5"))
    t_lo, t_hi, T = static["t_lo"], static["t_hi"], static["T"]

    def din(name, shape, dt):
        return nc.dram_tensor(name, shape, dt, kind="ExternalInput")

    idx_all = din("idx_all", [128, NW * T * 8], I16)
    dstl_all = din("dstl_all", [128, NW * T], BF16)
    nf_fm = din("nf_fm", [128, NLOC], BF16)
    ts_rep = din("ts_rep", [128, NLOC], F32)
    xp1_fm = din("xp1_fm", [128, NLOC], F32)
    xp1_fmb = din("xp1_fmb", [128, NLOC], BF16)
    xp2_fm = din("xp2_fm", [128, NLOC], F32)
    xp2_fmb = din("xp2_fmb", [128, NLOC], BF16)
    dinv_nm = din("dinv_nm", [128, NW], F32)
    dinvb_fm = din("dinvb_fm", [128, NLOC], BF16)
    mask_row = din("mask_row", [1, NLOC], BF16)

    cn = {}
    for nm, shape, dt in [
        ("R1a", [C, C], BF16), ("R1b", [C, C], BF16),
        ("S1a", [C, C], BF16), ("S1b", [C, C], BF16),
        ("W2T", [C, C], BF16),
        ("tab1_bias2", [2, C], BF16), ("skip_bias2", [2, C], BF16),
        ("b1_col", [C, 1], F32), ("b2_col", [C, 1], F32),
        ("iota", [128, 128], BF16),
        ("freq2_col", [C, 1], F32), ("pc_col", [C, 1], F32),
    ]:
        cn[nm] = din(nm, shape, dt)
    for l in (1, 2):
        for gate in "rzn":
            cn[f"g{l}Wi{gate}"] = din(f"g{l}Wi{gate}", [C, C], BF16)
            cn[f"g{l}Wh{gate}"] = din(f"g{l}Wh{gate}", [C, C], BF16)
        for nm in ("brz_r", "brz_z", "bin", "bhn"):
            cn[f"g{l}{nm}"] = din(f"g{l}{nm}", [C, 1], F32)

    h1_fm = nc.dram_tensor("h1_fm", [C, NLOC], F32, kind="ExternalOutput")
    h2_fm = nc.dram_tensor("h2_fm", [C, NLOC], F32, kind="ExternalOutput")

    tab_loc = [nc.dram_tensor(f"tab{l}_loc", [NLOC, C], BF16) for l in (1, 2)]
    tab_full = [nc.dram_tensor(f"tab{l}_full", [NV, C], BF16,
                               addr_space="Shared") for l in (1, 2)]
    tab_gat_lo = [nc.dram_tensor(f"tab{l}_glo", [HALF, C], BF16)
                  for l in (1, 2)]
    tab_gat_hi = [nc.dram_tensor(f"tab{l}_ghi", [NV - HALF, C], BF16)
                  for l in (1, 2)]
    bn_in = nc.dram_tensor("bn_in", [128, 2], F32)
    bn_out = nc.dram_tensor("bn_out", [128, 2], F32, addr_space="Shared")

    RG = [list(range(NCORES))]
    qsem = [nc.alloc_semaphore(f"gq{i}") for i in range(4)] \
        if GATHER_PREP else None
    qtgt = [0, 0, 0, 0]   # cumulative DMA-sem targets per queue

    with tile.TileContext(nc) as tc:
        res_cm = tc.tile_pool(name="res", bufs=1)
        res = res_cm.__enter__()

        # ---- resident tiles (per-partition bytes in comments) ----
        dstl_t = res.tile([128, NW * T], BF16, name="dstl_t")     # 3.3K
        nc.sync.dma_start(dstl_t[:], dstl_all[:])
        dstl_f = None
        if OHTS:
            dstl_f = res.tile([128, NW * T], F32, name="dstl_f")  # 6.7K
            nc.vector.tensor_copy(dstl_f[:], dstl_t[:])
        nf_t = res.tile([128, NLOC], BF16, name="nf_t")           # 12.25K
        nc.sync.dma_start(nf_t[:], nf_fm[:])
        te_t = res.tile([128, NLOC], BF16, name="te_t")           # 12.25K
        dinvb_t = res.tile([128, NLOC], BF16, name="dinvb_t")     # 12.25K
        nc.sync.dma_start(dinvb_t[:], dinvb_fm[:])
        dinv_nm_t = res.tile([128, NW], F32, name="dinv_nm_t")    # 0.2K
        nc.sync.dma_start(dinv_nm_t[:], dinv_nm[:])
        H1b_t = res.tile([128, NLOC], BF16, name="H1b_t")         # 12.25K
        Hcb_t = None
        if not FUSE:
            Hcb_t = res.tile([128, NLOC], BF16, name="Hcb_t")     # 12.25K
        Hpre_t = res.tile([128, NLOC], F32, name="Hpre_t")        # 24.5K

        w_t = {}
        for nm in cn:                                             # ~8K
            shape = list(cn[nm].shape)
            w_t[nm] = res.tile(shape, cn[nm].dtype, name=f"w_{nm}")
            nc.sync.dma_start(w_t[nm][:], cn[nm][:])
        ones2 = res.tile([2, 512], BF16, name="ones2")
        nc.vector.memset(ones2[:], 1.0)
        msum = res.tile([128, 2], F32, name="msum")
        bnred = res.tile([128, 2], F32, name="bnred")
        mean_c = res.tile([128, 1], F32, name="mean_c")
        istd_c = res.tile([128, 1], F32, name="istd_c")

        # ============ phase 1: t_embed = cos(ts*freq + phase) ============
        # z = ts*freq2 + (phase2 + 0.25); r0 = z - float(int(z)); the
        # f32->i32 DVE cast semantics (trunc vs round) decide the final
        # fold; K_COSV picks the variant validated on hardware.
        COSV = int(os.environ.get("K_COSV", "1"))
        CH = 784
        with tc.tile_pool(name="te", bufs=2) as tp:
            for lo in range(0, NLOC, CH):
                cs = slice(lo, lo + CH)
                tsr = tp.tile([128, CH], F32, name="tsr", tag="tsr")
                nc.sync.dma_start(tsr[:], ts_rep[:, cs])
                z = tp.tile([128, CH], F32, name="z", tag="z")
                nc.vector.tensor_scalar(z[:], tsr[:], w_t["freq2_col"][:],
                                        w_t["pc_col"][:],
                                        op0=AL.mult, op1=AL.add)
                zi = tp.tile([128, CH], I32, name="zi", tag="zi")
                nc.vector.tensor_copy(zi[:], z[:])
                zf = tp.tile([128, CH], F32, name="zf", tag="zf")
                nc.vector.tensor_copy(zf[:], zi[:])
                r = tp.tile([128, CH], F32, name="r", tag="r")
                if COSV == 1:
                    # cast rounds to nearest: r = z - zf in [-0.5, 0.5]
                    nc.vector.tensor_tensor(r[:], z[:], zf[:],
                                            op=AL.subtract)
                    nc.scalar.activation(te_t[:, cs], r[:], AF.Sin,
                                         bias=0.0, scale=2.0 * np.pi)
                else:
                    # cast truncates (z>0): r = (z-0.5) - zf in [-0.5, 0.5)
                    nc.vector.scalar_tensor_tensor(r[:], z[:], -0.5, zf[:],
                                                   op0=AL.add,
                                                   op1=AL.subtract)
                    nc.scalar.activation(te_t[:, cs], r[:], AF.Sin,
                                         bias=0.0, scale=-2.0 * np.pi)

        # ============ table production helpers ============
        def publish_table(l):
            nc.gpsimd.collective_compute(
                "AllGather", AL.bypass, replica_groups=RG,
                ins=[tab_loc[l - 1][:]], outs=[tab_full[l - 1][:]])
            if not DIRECT:
                nc.sync.dma_start(tab_gat_lo[l - 1][:],
                                  tab_full[l - 1][0:HALF, :])
                nc.sync.dma_start(tab_gat_hi[l - 1][:],
                                  tab_full[l - 1][HALF:NV, :])

        def make_table(l, produce):
            with tc.tile_pool(name=f"tab{l}", bufs=3) as tpp, \
                 tc.tile_pool(name=f"tab{l}ps", bufs=3, space="PSUM") as tps:
                for t in range(NW):
                    ts_ = slice(t * 128, (t + 1) * 128)
                    pt = tps.tile([128, 128], F32, name="pt", tag="pt")
                    produce(pt, ts_)
                    ot = tpp.tile([128, 128], BF16, name="ot", tag="ot")
                    nc.vector.tensor_scalar(ot[:], pt[:],
                                            dinv_nm_t[:, t:t + 1],
                                            None, op0=AL.mult)
                    nc.sync.dma_start(tab_loc[l - 1][ts_, :], ot[:])
            publish_table(l)

        def prod1(pt, ts_):
            nc.tensor.matmul(pt[:], nf_t[:, ts_], w_t["R1a"][:],
                             start=True, stop=False)
            nc.tensor.matmul(pt[:], te_t[:, ts_], w_t["R1b"][:],
                             start=False, stop=False)
            nc.tensor.matmul(pt[:], ones2[:, 0:128], w_t["tab1_bias2"][:],
                             start=False, stop=True)

        make_table(1, prod1)

        # ============ conv ============
        def conv(l, b_col, after_window=None):
            if DIRECT:
                tflo = tab_full[l - 1][0:HALF, :]
                tfhi = tab_full[l - 1][HALF:NV, :]
            else:
                tflo = tab_gat_lo[l - 1][:]
                tfhi = tab_gat_hi[l - 1][:]
            pending = [0, 0, 0, 0]
            wtgt = {}

            with tc.tile_pool(name=f"cv{l}", bufs=1) as gp, \
                 tc.tile_pool(name=f"cv{l}ps", bufs=3, space="PSUM") as cps:

                def issue_gather(w):
                    q_lo, q_hi = w % 4, (w + 2) % 4
                    # scalar-engine DGE queue: keeps the tiny idx loads off
                    # the Sync queue, which stalls behind the table copies
                    it_lo = gp.tile([128, t_lo * 8], I16, name="it_lo",
                                    tag="it_lo", bufs=PRE + 2)
                    nc.scalar.dma_start(
                        it_lo[:],
                        idx_all[:, w * T * 8:w * T * 8 + t_lo * 8])
                    it_hi = gp.tile([128, t_hi * 8], I16, name="it_hi",
                                    tag="it_hi", bufs=PRE + 2)
                    nc.scalar.dma_start(
                        it_hi[:],
                        idx_all[:, w * T * 8 + t_lo * 8:(w + 1) * T * 8])
                    glo = gp.tile([128, t_lo, 128], BF16, name="glo",
                                  tag="glo", bufs=PRE)
                    ghi = gp.tile([128, t_hi, 128], BF16, name="ghi",
                                  tag="ghi", bufs=PRE)
                    if NEGPAD and w < PRE:
                        # first use of each rotating buffer: clear stale
                        # SBUF so trimmed trailing slots can't inject NaN
                        nc.vector.memset(glo[:], 0.0)
                        nc.vector.memset(ghi[:], 0.0)
                    if GATHER_PREP:
                        nc.gpsimd.dma_gather(
                            glo[:], tflo, it_lo[:],
                            t_lo * 128, t_lo * 128, 128,
                            single_packet=False, queue_num=q_lo,
                            prepare_only=True, sem=qsem[q_lo])
                        pending[q_lo] += 1
                        qtgt[q_lo] += 16
                        nc.gpsimd.dma_gather(
                            ghi[:], tfhi, it_hi[:],
                            t_hi * 128, t_hi * 128, 128,
                            single_packet=False, queue_num=q_hi,
                            prepare_only=True, sem=qsem[q_hi])
                        pending[q_hi] += 1
                        qtgt[q_hi] += 16
                        wtgt[w] = (q_lo, qtgt[q_lo], q_hi, qtgt[q_hi])
                    else:
                        nc.gpsimd.dma_gather(
                            glo[:], tflo, it_lo[:],
                            t_lo * 128, t_lo * 128, 128,
                            single_packet=False, queue_num=q_lo)
                        nc.gpsimd.dma_gather(
                            ghi[:], tfhi, it_hi[:],
                            t_hi * 128, t_hi * 128, 128,
                            single_packet=False, queue_num=q_hi)
                    return glo, ghi

                def trigger_wave():
                    for q in range(4):
                        if pending[q]:
                            nc.gpsimd.trigger_dma(count=None, queue_num=q)
                            pending[q] = 0

                tiles = {}
                for w in range(min(PRE, NW)):
                    tiles[w] = issue_gather(w)

                hcb = None
                for w in range(NW):
                    if GATHER_PREP:
                        trigger_wave()
                    if FUSE and w % 4 == 0:
                        # per-chunk exchange tile: keeps conv->GRU handoff
                        # WAR hazards chunk-local instead of whole-tensor
                        hcb = gp.tile([128, 512], BF16, name="hcb",
                                      tag="hcb", bufs=3)
                    glo, ghi = tiles.pop(w)
                    oh = gp.tile([128, T, 128], BF16, name="oh",
                                 tag="oh", bufs=2)
                    if OHTS:
                        # per-tile tensor_scalar (1-source): leaves the
                        # DVE's shared read port free for Q7 desc-gen
                        for t in range(T):
                            nc.vector.tensor_scalar(
                                oh[:, t, :], w_t["iota"][:],
                                dstl_f[:, w * T + t:w * T + t + 1],
                                None, op0=AL.is_equal)
                    else:
                        dl = dstl_t[:, w * T:(w + 1) * T]
                        nc.vector.tensor_tensor(
                            oh[:], _bcast_free(dl, T, 128, "rep_elem"),
                            _bcast_free(w_t["iota"][:], T, 128, "rep_row"),
                            op=AL.is_equal)
                    ps = cps.tile([128, 128], F32, name="ps", tag="ps")
                    for t in range(t_lo):
                        nc.tensor.matmul(ps[:], glo[:, t, :], oh[:, t, :],
                                         start=(t == 0), stop=False)
                    for t in range(t_hi):
                        nc.tensor.matmul(ps[:], ghi[:, t, :],
                                         oh[:, t_lo + t, :],
                                         start=False, stop=(t == t_hi - 1))
                    ws = slice(w * 128, (w + 1) * 128)
                    td = gp.tile([128, 128], F32, name="td", tag="td",
                                 bufs=3)
                    nc.vector.tensor_tensor(td[:], ps[:], dinvb_t[:, ws],
                                            op=AL.mult)
                    if FUSE:
                        hs = slice((w % 4) * 128, (w % 4) * 128 + 128)
                        nc.scalar.activation(hcb[:, hs], td[:], AF.Identity,
                                             bias=b_col)
                    else:
                        nc.scalar.activation(Hcb_t[:, ws], td[:],
                                             AF.Identity, bias=b_col)
                    if w + PRE < NW:
                        tiles[w + PRE] = issue_gather(w + PRE)
                    if after_window is not None:
                        after_window(w, hcb)
                if GATHER_PREP:
                    trigger_wave()

        # ============ GRU chunk (emitted inline from the conv loop) =======
        def gru_chunk(l, gp, gps, xb_holder, xb_off, xf_dram, xfb_dram,
                      finish, off, n):
            cs = slice(off, off + n)
            xb_tile = xb_holder[:, xb_off:xb_off + n]
            xf = gp.tile([128, 512], F32, name="xf", tag="xf", bufs=2)
            nc.scalar.dma_start(xf[:, :n], xf_dram[:, cs])
            xfb = gp.tile([128, 512], BF16, name="xfb", tag="xfb", bufs=2)
            nc.scalar.dma_start(xfb[:, :n], xfb_dram[:, cs])

            def mm2(wi, wh):
                pi = gps.tile([128, 512], F32, name="pi", tag="pi", bufs=2)
                nc.tensor.matmul(pi[:, :n], w_t[wi][:], xb_tile,
                                 start=True, stop=False)
                nc.tensor.matmul(pi[:, :n], w_t[wh][:], xfb[:, :n],
                                 start=False, stop=True)
                return pi

            smr = mm2(f"g{l}Wir", f"g{l}Whr")
            r = gp.tile([128, 512], F32, name="r", tag="r", bufs=2)
            nc.scalar.activation(r[:, :n], smr[:, :n], AF.Sigmoid,
                                 bias=w_t[f"g{l}brz_r"][:])
            smz = mm2(f"g{l}Wiz", f"g{l}Whz")
            z = gp.tile([128, 512], F32, name="z", tag="z", bufs=2)
            nc.scalar.activation(z[:, :n], smz[:, :n], AF.Sigmoid,
                                 bias=w_t[f"g{l}brz_z"][:])
            pin = gps.tile([128, 512], F32, name="pin", tag="pi", bufs=2)
            nc.tensor.matmul(pin[:, :n], w_t[f"g{l}Win"][:],
                             xb_tile, start=True, stop=True)
            phn = gps.tile([128, 512], F32, name="phn", tag="ph", bufs=1)
            nc.tensor.matmul(phn[:, :n], w_t[f"g{l}Whn"][:],
                             xfb[:, :n], start=True, stop=True)
            # rn = (phn + bhn) * r  in one DVE pass
            rn = gp.tile([128, 512], F32, name="rn", tag="rn", bufs=2)
            nc.vector.scalar_tensor_tensor(
                rn[:, :n], phn[:, :n], w_t[f"g{l}bhn"][:], r[:, :n],
                op0=AL.add, op1=AL.mult)
            t2 = gp.tile([128, 512], F32, name="t2", tag="t2", bufs=2)
            nc.vector.tensor_tensor(t2[:, :n], pin[:, :n], rn[:, :n],
                                    op=AL.add)
            ng = gp.tile([128, 512], F32, name="ng", tag="ng", bufs=2)
            nc.scalar.activation(ng[:, :n], t2[:, :n], AF.Tanh,
                                 bias=w_t[f"g{l}bin"][:])
            d = gp.tile([128, 512], F32, name="d", tag="d", bufs=2)
            nc.vector.tensor_tensor(d[:, :n], xf[:, :n], ng[:, :n],
                                    op=AL.subtract)
            zd = gp.tile([128, 512], F32, name="zd", tag="zd", bufs=2)
            nc.vector.tensor_tensor(zd[:, :n], z[:, :n], d[:, :n],
                                    op=AL.mult)
            H = gp.tile([128, 512], F32, name="H", tag="H", bufs=2)
            nc.vector.tensor_tensor(H[:, :n], ng[:, :n], zd[:, :n],
                                    op=AL.add)
            finish(gp, gps, H, off, n)

        # --- GRU1 finish: relu -> h1 out + H1 bf16 + table2 tiles ---
        def fin1(gp, gps, H, off, n):
            cs = slice(off, off + n)
            Hr = gp.tile([128, 512], F32, name="Hr", tag="Hr", bufs=3)
            nc.scalar.activation(Hr[:, :n], H[:, :n], AF.Relu, bias=0.0)
            nc.sync.dma_start(h1_fm[:, cs], Hr[:, :n])
            nc.vector.tensor_copy(H1b_t[:, cs], Hr[:, :n])
            for t in range(off // 128, (off + n) // 128):
                ts_ = slice(t * 128, (t + 1) * 128)
                pt = gps.tile([128, 128], F32, name="pt", tag="pt", bufs=1)
                nc.tensor.matmul(pt[:], H1b_t[:, ts_], w_t["W2T"][:],
                                 start=True, stop=True)
                ot = gp.tile([128, 128], BF16, name="ot", tag="ot", bufs=2)
                nc.vector.tensor_scalar(ot[:], pt[:], dinv_nm_t[:, t:t + 1],
                                        None, op0=AL.mult)
                nc.sync.dma_start(tab_loc[1][ts_, :], ot[:])

        # --- GRU2 finish: + skip -> Hpre ---
        def fin2(gp, gps, H, off, n):
            cs = slice(off, off + n)
            pk = gps.tile([128, 512], F32, name="pk", tag="pk", bufs=1)
            nc.tensor.matmul(pk[:, :n], w_t["S1a"][:], nf_t[:, cs],
                             start=True, stop=False)
            nc.tensor.matmul(pk[:, :n], w_t["S1b"][:], te_t[:, cs],
                             start=False, stop=False)
            nc.tensor.matmul(pk[:, :n], w_t["skip_bias2"][:], ones2[:, :n],
                             start=False, stop=True)
            nc.vector.tensor_tensor(Hpre_t[:, cs], H[:, :n], pk[:, :n],
                                    op=AL.add)

        # ---- conv1 (+ GRU1: fused in, or as its own phase) ----
        if PH >= 2:
            if FUSE:
                with tc.tile_pool(name="gru1", bufs=1) as g1p, \
                     tc.tile_pool(name="gru1ps", bufs=1,
                                  space="PSUM") as g1ps:
                    def after_w1(w, hcb):
                        if PH < 3:
                            return
                        if (w + 1) % 4 == 0:
                            j = (w + 1) // 4 - 1
                            gru_chunk(1, g1p, g1ps, hcb, 0, xp1_fm,
                                      xp1_fmb, fin1, j * 512, 512)
                        elif w == NW - 1:
                            gru_chunk(1, g1p, g1ps, hcb, 0, xp1_fm,
                                      xp1_fmb, fin1, (w // 4) * 512,
                                      NLOC - (w // 4) * 512)
                    conv(1, w_t["b1_col"][:], after_w1)
            else:
                conv(1, w_t["b1_col"][:], None)
                if PH >= 3:
                    with tc.tile_pool(name="gru1", bufs=1) as g1p, \
                         tc.tile_pool(name="gru1ps", bufs=1,
                                      space="PSUM") as g1ps:
                        for off in range(0, NLOC, 512):
                            n = min(512, NLOC - off)
                            gru_chunk(1, g1p, g1ps, Hcb_t, off, xp1_fm,
                                      xp1_fmb, fin1, off, n)
        if PH >= 3:
            publish_table(2)
        else:
            nc.vector.memset(H1b_t[:], 0.0)
            z1 = res.tile([128, 512], F32, name="z1")
            nc.vector.memset(z1[:], 0.0)
            for off in range(0, NLOC, 512):
                n = min(512, NLOC - off)
                nc.sync.dma_start(h1_fm[:, off:off + n], z1[:, :n])

        # ---- conv2 (+ GRU2: fused in, or as its own phase) ----
        if PH >= 4:
            if FUSE:
                with tc.tile_pool(name="gru2", bufs=1) as g2p, \
                     tc.tile_pool(name="gru2ps", bufs=1,
                                  space="PSUM") as g2ps:
                    def after_w2(w, hcb):
                        if PH < 5:
                            return
                        if (w + 1) % 4 == 0:
                            j = (w + 1) // 4 - 1
                            gru_chunk(2, g2p, g2ps, hcb, 0, xp2_fm,
                                      xp2_fmb, fin2, j * 512, 512)
                        elif w == NW - 1:
                            gru_chunk(2, g2p, g2ps, hcb, 0, xp2_fm,
                                      xp2_fmb, fin2, (w // 4) * 512,
                                      NLOC - (w // 4) * 512)
                    conv(2, w_t["b2_col"][:], after_w2)
            else:
                conv(2, w_t["b2_col"][:], None)
                if PH >= 5:
                    with tc.tile_pool(name="gru2", bufs=1) as g2p, \
                         tc.tile_pool(name="gru2ps", bufs=1,
                                      space="PSUM") as g2ps:
                        for off in range(0, NLOC, 512):
                            n = min(512, NLOC - off)
                            gru_chunk(2, g2p, g2ps, Hcb_t, off, xp2_fm,
                                      xp2_fmb, fin2, off, n)
        if PH < 5:
            nc.vector.memset(Hpre_t[:], 0.0)

        # ============ BatchNorm ============
        with tc.tile_pool(name="bn", bufs=1) as bp, \
             tc.tile_pool(name="bnps", bufs=2, space="PSUM") as bps:
            mask_t2 = bp.tile([1, NLOC], BF16, name="mask_t2")
            nc.sync.dma_start(mask_t2[:], mask_row[:])
            nchunk = (NLOC + 511) // 512
            part_s = bp.tile([128, nchunk], F32, name="part_s")
            part_q = bp.tile([128, nchunk], F32, name="part_q")
            for ci, off in enumerate(range(0, NLOC, 512)):
                n = min(512, NLOC - off)
                cs = slice(off, off + n)
                pm = bps.tile([128, 512], F32, name="pm", tag="pm", bufs=2)
                nc.tensor.matmul(pm[:, :n], ones2[0:1, 0:128],
                                 mask_t2[:, cs], start=True, stop=True)
                hm = bp.tile([128, 512], F32, name="hm", tag="hm", bufs=2)
                nc.vector.tensor_tensor(hm[:, :n], Hpre_t[:, cs], pm[:, :n],
                                        op=AL.mult)
                nc.vector.tensor_reduce(part_s[:, ci:ci + 1], hm[:, :n],
                                        axis=mybir.AxisListType.X, op=AL.add)
                sqs = bp.tile([128, 512], F32, name="sqs", tag="sqs", bufs=2)
                nc.scalar.activation(sqs[:, :n], hm[:, :n], AF.Square,
                                     bias=0.0,
                                     accum_out=part_q[:, ci:ci + 1])
            nc.vector.tensor_reduce(msum[:, 0:1], part_s[:],
                                    axis=mybir.AxisListType.X, op=AL.add)
            nc.vector.tensor_reduce(msum[:, 1:2], part_q[:],
                                    axis=mybir.AxisListType.X, op=AL.add)
            nc.sync.dma_start(bn_in[:], msum[:])
            nc.gpsimd.collective_compute(
                "AllReduce", AL.add, replica_groups=RG,
                ins=[bn_in[:]], outs=[bn_out[:]])
            nc.sync.dma_start(bnred[:], bn_out[:])
            nc.vector.tensor_scalar(mean_c[:], bnred[:, 0:1], 1.0 / N, None,
                                    op0=AL.mult)
            m2 = bp.tile([128, 1], F32, name="m2")
            nc.vector.tensor_tensor(m2[:], mean_c[:], mean_c[:], op=AL.mult)
            v1 = bp.tile([128, 1], F32, name="v1")
            nc.vector.tensor_scalar(v1[:], bnred[:, 1:2], 1.0 / N, None,
                                    op0=AL.mult)
            v2 = bp.tile([128, 1], F32, name="v2")
            nc.vector.tensor_tensor(v2[:], v1[:], m2[:], op=AL.subtract)
            v3 = bp.tile([128, 1], F32, name="v3")
            nc.vector.tensor_scalar(v3[:], v2[:], 1e-5, None, op0=AL.add)
            v4 = bp.tile([128, 1], F32, name="v4")
            nc.scalar.activation(v4[:], v3[:], AF.Sqrt, bias=0.0)
            nc.vector.reciprocal(istd_c[:], v4[:])
            for off in range(0, NLOC, 512):
                n = min(512, NLOC - off)
                hn_ = bp.tile([128, 512], F32, name="hn_", tag="hn_", bufs=3)
                nc.vector.tensor_scalar(hn_[:, :n], Hpre_t[:, off:off + n],
                                        mean_c[:], istd_c[:],
                                        op0=AL.subtract, op1=AL.mult)
                nc.sync.dma_start(h2_fm[:, off:off + n], hn_[:, :n])

        res_cm.__exit__(None, None, None)
    return nc


# ---------------------------------------------------------------------------
# entry point
# ---------------------------------------------------------------------------

def _install_ntff_hook():
    """Install antenv.axon_hooks (missing in this image) for trace=True."""
    import sys
    import types
    try:
        import antenv
        if getattr(antenv, "axon_hooks", None) is not None:
            return
        from trn_agent_boot.trn_boot import _ntff_profile_via_ctypes
        hook = _ntff_profile_via_ctypes("/opt/axon/libaxon_pjrt.so")
        mod = types.ModuleType("antenv.axon_hooks")
        mod.set_axon_ntff_profile_hook = lambda h: None
        mod.get_axon_ntff_profile_hook = lambda: hook
        sys.modules["antenv.axon_hooks"] = mod
        antenv.axon_hooks = mod
    except Exception:
        pass


def kernel(**inputs):
    global LAST_EXEC_NS
    per_core, static = _preprocess(inputs)

    nc = bacc.Bacc("TRN2", target_bir_lowering=False, debug=False,
                   num_devices=NCORES, num_swdge_queues=4)
    _build(nc, static)
    nc.compile()

    in_maps = [per_core[k] for k in range(NCORES)]
    trace = os.environ.get("KERNEL_TRACE", "0") == "1"
    if trace:
        _install_ntff_hook()
    res = run_bass_kernel_spmd(nc, in_maps, list(range(NCORES)), trace=trace)
    LAST_EXEC_NS = res.exec_time_ns

    H1 = np.zeros((N, C), np.float32)
    H2 = np.zeros((N, C), np.float32)
    for k in range(NCORES):
        lo, hi_ = k * NLOC, min((k + 1) * NLOC, N)
        if lo >= N:
            break
        nrow = hi_ - lo
        H1[lo:hi_] = res.results[k]["h1_fm"].T[:nrow]
        H2[lo:hi_] = res.results[k]["h2_fm"].T[:nrow]
    return (H1, H2)
